# revision 1
# baseline (speedup 1.0000x reference)
"""DiffFDN Trainium2 kernel, v3: DRAM history + indirect gathers.

Per core (4 items): the 48000-step FDN scan becomes 94 blocks of
[64x68]^T @ [64x500] (float32r). History lives in DRAM as one
[68, TPAD] tensor (rows 0-63 per-(line,item) nxt series, rows 64-67 the
y output series). Per block: one PSUM->SBUF copy, one staged HWDGE
write to DRAM, one SWDGE *indirect* gather (per-row flat offsets) that
assembles the 16 time-shifted line reads in a single instruction.

The gather's in_ AP is the column-sliced prefix H[:, 0:PAD+n0-500] so
the Tile dependency tracker sees the true dependency (the write from
block b-2), keeping a 2-deep software pipeline; offsets are view-flat
element indices r*X_b + (PAD + n0 - d_i).
"""

import numpy as np

SR = 48000
IR_LEN = 48000
DELAYS = [1009, 1123, 1231, 1321, 1433, 1543, 1657, 1777, 1879, 1987,
          2081, 2179, 2287, 2383, 2503, 2617]
N = 16
FEAT = 256
BATCH = 32
NCORES = 8
IPC = BATCH // NCORES
L = 500
PAD = 2620                     # zero padding before t=0 (>= max delay)
TPAD = PAD + IR_LEN + 500
NBLK = IR_LEN // L             # 96; blocks 0,1 skipped (all-zero)
M_OUT = IPC * N + IPC          # 68

_BUILT = None
K_DEP = 2          # gather only covers blocks <= b-K_DEP (pipeline depth)


def _patch_list():
    """Pieces of each line's read window sourced from blocks > b-K_DEP.

    These are patched into S from the SBUF stage tiles (the DRAM gather
    raced/skipped those regions). Returns (line, rel_blk, src_col,
    dst_col, length) tuples; rel_blk is source block minus b.
    """
    out = []
    for i in range(N):
        d = DELAYS[i]
        lo, hi = -d, -d + L
        a = lo
        while a < hi:
            e = min(hi, (a // L + 1) * L)
            rel = a // L
            if rel >= -(K_DEP - 1):
                out.append((i, rel, a - rel * L, a - lo, e - a))
            a = e
    return out


def _expm64(M):
    M = M.astype(np.float64)
    nrm = np.linalg.norm(M, ord=np.inf)
    k = max(0, int(np.ceil(np.log2(max(nrm, 1e-30)))) + 2)
    Ms = M / (2.0 ** k)
    E = np.eye(M.shape[0]) + Ms
    term = Ms.copy()
    for i in range(2, 18):
        term = term @ Ms / i
        E = E + term
    for _ in range(k):
        E = E @ E
    return E


def _prologue(x, WA, bA, WB, bB, WC, bC):
    x = np.asarray(x, np.float32)
    feat = x.mean(axis=1)
    A = np.tanh(feat @ np.asarray(WA).T + bA).reshape(-1, N, N)
    Bv = np.tanh(feat @ np.asarray(WB).T + bB)
    Cv = np.tanh(feat @ np.asarray(WC).T + bC)
    S = np.triu(A, 1)
    S = S - np.swapaxes(S, -1, -2)
    g = 10.0 ** (-3.0 / SR)
    G = g ** np.asarray(DELAYS, np.float64)
    A_g = np.stack([_expm64(S[b]) for b in range(S.shape[0])])
    A_g = (A_g * G[None, None, :]).astype(np.float32)
    return A_g, Bv.astype(np.float32), Cv.astype(np.float32)


def _core_inputs(A_g4, Bv4, Cv4):
    lhsT = np.zeros((IPC * N, M_OUT), np.float32)
    bv = np.zeros((IPC * N, 1), np.float32)
    for j in range(IPC):
        for i in range(N):
            r = 4 * i + j
            for ip in range(N):
                lhsT[r, 4 * ip + j] = A_g4[j, ip, i]
            lhsT[r, IPC * N + j] = Cv4[j, i]
            bv[r, 0] = Bv4[j, i]
    return lhsT, bv


OFFS_PHYSICAL = True


def _offsets():
    """offs[r, b-2] = flat gather offset for row r, sub-block b.

    OFFS_PHYSICAL: offsets are element offsets into the physical tensor
    (row stride TPAD) -- what the HW descriptor generator uses. CoreSim
    instead flattens the sliced view (row stride X_b).
    """
    offs = np.zeros((IPC * N, NBLK - 2), np.uint32)
    for b in range(2, NBLK):
        n0 = L * b
        Xb = TPAD if OFFS_PHYSICAL else (PAD + n0 - L)
        for i in range(N):
            for j in range(IPC):
                r = 4 * i + j
                offs[r, b - 2] = r * Xb + (PAD + n0 - DELAYS[i])
    return offs


def _build():
    global _BUILT
    if _BUILT is not None:
        return _BUILT
    import concourse.bacc as bacc
    import concourse.bass as bass
    import concourse.mybir as mybir
    import concourse.tile as tile

    fp32 = mybir.dt.float32
    f32r = mybir.dt.float32r
    u32 = mybir.dt.uint32
    nc = bacc.Bacc("TRN2", target_bir_lowering=False, debug=False)
    lhsT_d = nc.dram_tensor("lhsT", [IPC * N, M_OUT], f32r, kind="ExternalInput")
    bv_d = nc.dram_tensor("bv", [IPC * N, 1], f32r, kind="ExternalInput")
    offs_d = nc.dram_tensor("offs", [IPC * N, NBLK - 2], u32, kind="ExternalInput")
    patches = _patch_list()
    npat = len(patches)
    pmask_d = None
    if npat:
        pmask_d = nc.dram_tensor(
            "pmask", [IPC * N, npat], mybir.dt.uint8, kind="ExternalInput")
    y_d = nc.dram_tensor("y", [IPC, IR_LEN], f32r, kind="ExternalOutput")
    h_d = nc.dram_tensor("hist", [M_OUT, TPAD], f32r)

    with tile.TileContext(nc) as tc:
        with tc.tile_pool(name="const", bufs=1) as cpool, \
             tc.tile_pool(name="init", bufs=1) as ipool, \
             tc.tile_pool(name="sg", bufs=8) as spool, \
             tc.tile_pool(name="st", bufs=10) as tpool, \
             tc.tile_pool(name="ps", bufs=8, space="PSUM") as ppool, \
             tc.tile_pool(name="yb", bufs=2) as ypool:
            lhsT = cpool.tile([IPC * N, M_OUT], f32r)
            nc.sync.dma_start(lhsT[:, :], lhsT_d[:, :])
            offs = cpool.tile([IPC * N, NBLK - 2], u32)
            nc.sync.dma_start(offs[:, :], offs_d[:, :])
            pmask = None
            if npat:
                pmask = cpool.tile([IPC * N, npat], mybir.dt.uint8)
                nc.sync.dma_start(pmask[:, :], pmask_d[:, :])

            # zero-init history cols [0, PAD+1000) incl. y rows; Bv impulse
            # lands at col PAD (time 0) via the same staged image.
            z = ipool.tile([M_OUT, PAD + 2 * L], fp32)
            half = (PAD + 2 * L) // 2
            nc.vector.memset(z[:, 0:half], 0.0)
            nc.gpsimd.memset(z[:, half:], 0.0)
            nc.sync.dma_start(z[0:IPC * N, PAD:PAD + 1].bitcast(f32r), bv_d[:, :])
            nc.scalar.dma_start(
                h_d[:, 0:PAD + 2 * L].bitcast(fp32), z[:, :])

            stages = {}  # b -> (tile, col0) holding that block's nxt in SBUF
            stages[-1] = (z, PAD - L)   # negative time: zeros
            stages[0] = (z, PAD)
            stages[1] = (z, PAD + L)
            for b in range(2, NBLK):
                n0 = L * b
                Xb = PAD + n0 - (K_DEP - 1) * L
                S = spool.tile([IPC * N, L], f32r)
                nc.gpsimd.indirect_dma_start(
                    out=S[:, :], out_offset=None,
                    in_=h_d[0:IPC * N, 0:Xb],
                    in_offset=bass.IndirectOffsetOnAxis(
                        ap=offs[:, b - 2:b - 1], axis=1),
                )
                # patch recent-sourced pieces of S from the SBUF stages
                # (the DRAM gather raced/skipped those regions). Engines
                # require 32-aligned partition bases, so each patch runs
                # base-0 over all rows with a per-line row mask.
                for k, (i, rel, sc, dc, ln) in enumerate(patches):
                    ptile, pcol = stages[b + rel]
                    src = ptile[0:IPC * N, pcol + sc:pcol + sc + ln]
                    if src.dtype != f32r:
                        src = src.bitcast(f32r)
                    nc.vector.copy_predicated(
                        S[:, dc:dc + ln],
                        pmask[:, k:k + 1].to_broadcast([IPC * N, ln]),
                        src,
                    )
                ps = ppool.tile([M_OUT, L], fp32)
                nc.tensor.matmul(ps[:, :], lhsT[:, :], S[:, :],
                                 start=True, stop=True)
                stage = tpool.tile([M_OUT, L], f32r)
                stages[b] = (stage, 0)
                if b % 2 == 0:
                    nc.vector.tensor_copy(stage[:, :], ps[:, :])
                else:
                    nc.scalar.copy(stage[:, :], ps[:, :])
                weng = nc.sync if b % 2 == 0 else nc.scalar
                weng.dma_start(h_d[:, PAD + n0:PAD + n0 + L], stage[:, :])

                # y extraction (hist rows 64..67 -> y, bounced via SBUF),
                # interleaved: chunk k is final once block 24*(k+1) has
                # been written, so it overlaps the remaining compute
                # instead of trailing the last block.
                CH = 12000
                if b >= 25 and (b - 25) % 24 == 0 and (k := (b - 25) // 24) < 3:
                    yb = ypool.tile([IPC, CH], f32r)
                    nc.scalar.dma_start(
                        yb[:, :],
                        h_d[IPC * N:M_OUT, PAD + k * CH:PAD + (k + 1) * CH])
                    nc.scalar.dma_start(y_d[:, k * CH:(k + 1) * CH], yb[:, :])
                # last two blocks: ship y straight from the SBUF stage so
                # the kernel tail doesn't wait on their DRAM writes
                if b >= NBLK - 2:
                    nc.sync.dma_start(
                        y_d[:, n0:n0 + L], stage[IPC * N:M_OUT, :])

            # remaining y span [3*CH, last two blocks) after the loop;
            # reads only blocks <= NBLK-3 so it overlaps the loop tail
            yb = ypool.tile([IPC, CH], f32r)
            span = IR_LEN - L * 2 - 3 * CH
            nc.scalar.dma_start(
                yb[:, 0:span],
                h_d[IPC * N:M_OUT, PAD + 3 * CH:PAD + 3 * CH + span])
            nc.scalar.dma_start(y_d[:, 3 * CH:3 * CH + span], yb[:, 0:span])
    nc.compile()
    _BUILT = nc
    return nc


def _pmask():
    pl = _patch_list()
    pm = np.zeros((IPC * N, len(pl)), np.uint8)
    for k, (i, _, _, _, _) in enumerate(pl):
        pm[4 * i:4 * i + 4, k] = 1
    return pm


def kernel(x, WA, bA, WB, bB, WC, bC):
    from concourse import bass_utils

    A_g, Bv, Cv = _prologue(x, WA, bA, WB, bB, WC, bC)
    offs = _offsets()
    pm = _pmask()
    in_maps = []
    for k in range(NCORES):
        sl = slice(k * IPC, (k + 1) * IPC)
        lhsT, bv = _core_inputs(A_g[sl], Bv[sl], Cv[sl])
        m = {"lhsT": lhsT, "bv": bv, "offs": offs}
        if pm.shape[1]:
            m["pmask"] = pm
        in_maps.append(m)

    nc = _build()
    res = bass_utils.run_bass_kernel_spmd(nc, in_maps, core_ids=list(range(NCORES)))
    y = np.concatenate([res.results[k]["y"] for k in range(NCORES)], axis=0)
    return y[:, None, :].astype(np.float32)



# revision 6
# speedup vs baseline: 1.3343x; 1.3343x over previous
"""DiffFDN Trainium2 kernel, v4: all-SBUF fp16 history + gpsimd local_scatter.

Per core (4 items, lockstep): the 48000-step FDN scan runs as 94 blocks of
L=504 timesteps. History lives entirely in SBUF as a doubled ring of 16
slots ([128, 8064] fp16, rows 0-63 nxt series, 64-67 the y series; slot s
holds block s mod 8, duplicated at slot+8 so any <=5-slot window is a
contiguous column slice). Realignment of the 16 per-line delay taps uses
InstLocalScatter (per-partition int16 index tables, Pool engine): per block
pair {b, b+1} one BIG scatter assembles everything sourced from stages
<= b-2 into SB [128,1008] and one SMALL scatter adds stage b-1's
contribution for block b+1 into SS [128,504]. Both outputs are zero-filled
off their written lanes, so two accumulating matmuls (lhsT^T @ SB-half +
lhsT^T @ SS) reconstruct the exact tap matrix by linearity. No DMA on the
recurrence chain; y drains from ring rows 64-67 via periodic HWDGE DMAs.
"""

import numpy as np

SR = 48000
IR_LEN = 48000
DELAYS = [1009, 1123, 1231, 1321, 1433, 1543, 1657, 1777, 1879, 1987,
          2081, 2179, 2287, 2383, 2503, 2617]
N = 16
FEAT = 256
BATCH = 32
NCORES = 8
IPC = BATCH // NCORES          # items per core
L = 504
NBLK = 96                      # blocks 0,1 are identically zero (d_min=1009)
RING = 8                       # ring slots (doubled to 16 in storage)
RW = RING * L                  # 4032, primary ring width
BIGW = 2118                    # BIG window [n0-2622, n0-504); even base for Q7
SMALLW = L                     # SMALL window [n0-504, n0) = stage b-1 exactly
M_OUT = IPC * N + IPC          # 68 psum rows (64 nxt + 4 y)

_BUILT = None


def _expm64(M):
    M = M.astype(np.float64)
    nrm = np.linalg.norm(M, ord=np.inf)
    k = max(0, int(np.ceil(np.log2(max(nrm, 1e-30)))) + 2)
    Ms = M / (2.0 ** k)
    E = np.eye(M.shape[0]) + Ms
    term = Ms.copy()
    for i in range(2, 18):
        term = term @ Ms / i
        E = E + term
    for _ in range(k):
        E = E @ E
    return E


def _prologue(x, WA, bA, WB, bB, WC, bC):
    x = np.asarray(x, np.float32)
    feat = x.mean(axis=1)
    A = np.tanh(feat @ np.asarray(WA).T + bA).reshape(-1, N, N)
    Bv = np.tanh(feat @ np.asarray(WB).T + bB)
    Cv = np.tanh(feat @ np.asarray(WC).T + bC)
    S = np.triu(A, 1)
    S = S - np.swapaxes(S, -1, -2)
    g = 10.0 ** (-3.0 / SR)
    G = g ** np.asarray(DELAYS, np.float64)
    A_g = np.stack([_expm64(S[b]) for b in range(S.shape[0])])
    A_g = (A_g * G[None, None, :]).astype(np.float32)
    return A_g, Bv.astype(np.float32), Cv.astype(np.float32)


def _core_inputs(A_g4, Bv4, Cv4):
    """lhsT [64, 68] fp16: contraction row r=4i+j (line i item j) ->
    psum rows m=4i'+j (nxt) and m=64+j (y)."""
    lhsT = np.zeros((IPC * N, M_OUT), np.float32)
    bv = np.zeros((IPC * N, 1), np.float32)
    for j in range(IPC):
        for i in range(N):
            r = 4 * i + j
            for ip in range(N):
                lhsT[r, 4 * ip + j] = A_g4[j, ip, i]
            lhsT[r, IPC * N + j] = Cv4[j, i]
            bv[r, 0] = Bv4[j, i]
    return lhsT.astype(np.float16), bv.astype(np.float16)


def _idx_tables():
    """Per-partition scatter tables (block-phase independent).

    BIG: data col k <-> time t = n0-2622+k; row r (line i_r) valid when
    dst = k - 2622 + d_r lands in [0, 1008) (covers block b cols 0-503 and
    block b+1 cols 504-1007, all sourced from stages <= b-2; the window's
    2B base must be 4B-aligned for the Q7 uint32-pair reads).
    SMALL: data col k <-> t = n0-504+k (stage b-1); dst = k + d_r - 1008
    in [0, 504) covers block b+1's tail."""
    idxb = np.full((128, BIGW), -1, np.int16)
    idxs = np.full((128, SMALLW), -1, np.int16)
    for i in range(N):
        d = DELAYS[i]
        for j in range(IPC):
            r = 4 * i + j
            for k in range(BIGW):
                dst = k - 2622 + d
                if 0 <= dst < 2 * L:
                    idxb[r, k] = dst
            for k in range(SMALLW):
                dst = k + d - 1008
                if 0 <= dst < L:
                    idxs[r, k] = dst
    return idxb, idxs


def _build():
    global _BUILT
    if _BUILT is not None:
        return _BUILT
    import concourse.bacc as bacc
    import concourse.mybir as mybir
    import concourse.tile as tile

    fp32 = mybir.dt.float32
    fp16 = mybir.dt.float16
    i16 = mybir.dt.int16
    nc = bacc.Bacc("TRN2", target_bir_lowering=False, debug=False)
    lhsT_d = nc.dram_tensor("lhsT", [IPC * N, M_OUT], fp16, kind="ExternalInput")
    bv_d = nc.dram_tensor("bv", [IPC * N, 1], fp16, kind="ExternalInput")
    idxb_d = nc.dram_tensor("idxb", [128, BIGW], i16, kind="ExternalInput")
    idxs_d = nc.dram_tensor("idxs", [128, SMALLW], i16, kind="ExternalInput")
    y_d = nc.dram_tensor("y", [IPC, NBLK * L], fp16, kind="ExternalOutput")

    with tile.TileContext(nc) as tc:
        with tc.tile_pool(name="const", bufs=1) as cpool, \
             tc.tile_pool(name="sb", bufs=4) as sbpool, \
             tc.tile_pool(name="ss", bufs=4) as sspool, \
             tc.tile_pool(name="ps", bufs=6, space="PSUM") as ppool:
            lhsT = cpool.tile([IPC * N, M_OUT], fp16)
            nc.sync.dma_start(lhsT[:, :], lhsT_d[:, :])
            idxb = cpool.tile([128, BIGW], i16)
            nc.sync.dma_start(idxb[:, :], idxb_d[:, :])
            idxs = cpool.tile([128, SMALLW], i16)
            nc.sync.dma_start(idxs[:, :], idxs_d[:, :])

            ring = cpool.tile([128, 2 * RW], fp16)
            nc.vector.memset(ring[:, 0:RW], 0.0)
            nc.gpsimd.memset(ring[:, RW:2 * RW], 0.0)
            # impulse: nxt(0) = Bv at t=0 (slot 0 col 0, both images)
            nc.sync.dma_start(ring[0:IPC * N, 0:1], bv_d[:, :])
            nc.sync.dma_start(ring[0:IPC * N, RW:RW + 1], bv_d[:, :])
            # y for blocks 0,1 is identically zero
            nc.sync.dma_start(y_d[:, 0:2 * L], ring[IPC * N:M_OUT, 0:2 * L])

            for b in range(2, NBLK):
                n0 = b * L
                ps = ppool.tile([M_OUT, L], fp32)
                if b % 2 == 0:
                    # BIG scatter for pair {b, b+1}: window starts at
                    # slot (b-6) col 402  <->  t = n0 - 2622 (4B-aligned)
                    base = ((b - 6) % RING) * L + 402
                    SB = sbpool.tile([128, 2 * L], fp16)
                    nc.gpsimd.local_scatter(
                        SB[:, :], ring[:, base:base + BIGW], idxb[:, :],
                        channels=128, num_elems=2 * L, num_idxs=BIGW)
                    nc.tensor.matmul(ps[:, :], lhsT[:, :], SB[0:IPC * N, 0:L],
                                     start=True, stop=True)
                else:
                    # SMALL scatter: data = ring slot (b-2) = stage b-1
                    # exactly (<-> t in [n0'-504, n0') for the pair's n0')
                    base = ((b - 2) % RING) * L
                    SS = sspool.tile([128, L], fp16)
                    nc.gpsimd.local_scatter(
                        SS[:, :], ring[:, base:base + SMALLW], idxs[:, :],
                        channels=128, num_elems=L, num_idxs=SMALLW)
                    nc.tensor.matmul(ps[:, :], lhsT[:, :], SB[0:IPC * N, L:2 * L],
                                     start=True, stop=False)
                    nc.tensor.matmul(ps[:, :], lhsT[:, :], SS[0:IPC * N, :],
                                     start=False, stop=True)
                # write block into both ring images (fp32 -> fp16 casts)
                slot = (b % RING) * L
                nc.vector.tensor_copy(ring[0:M_OUT, slot:slot + L], ps[:, :])
                nc.scalar.copy(ring[0:M_OUT, RW + slot:RW + slot + L], ps[:, :])
                # y drain: blocks [b-4, b-1] once their ring writes landed
                if b % 4 == 2 and b >= 6:
                    ybase = ((b - 4) % RING) * L
                    nc.sync.dma_start(
                        y_d[:, (b - 4) * L:b * L],
                        ring[IPC * N:M_OUT, ybase:ybase + 4 * L])
            # tail: blocks 94, 95 (ring slots 6, 7)
            nc.sync.dma_start(
                y_d[:, (NBLK - 2) * L:NBLK * L],
                ring[IPC * N:M_OUT, 6 * L:8 * L])
    nc.compile()
    _BUILT = nc
    return nc


def make_in_maps(inputs):
    A_g, Bv, Cv = _prologue(**inputs)
    idxb, idxs = _idx_tables()
    in_maps = []
    for k in range(NCORES):
        sl = slice(k * IPC, (k + 1) * IPC)
        lhsT, bv = _core_inputs(A_g[sl], Bv[sl], Cv[sl])
        in_maps.append({"lhsT": lhsT, "bv": bv, "idxb": idxb, "idxs": idxs})
    return in_maps


def kernel(x, WA, bA, WB, bB, WC, bC):
    from concourse import bass_utils

    in_maps = make_in_maps(
        {"x": x, "WA": WA, "bA": bA, "WB": WB, "bB": bB, "WC": WC, "bC": bC})
    nc = _build()
    res = bass_utils.run_bass_kernel_spmd(nc, in_maps, core_ids=list(range(NCORES)))
    y = np.concatenate(
        [res.results[k]["y"][:, :IR_LEN] for k in range(NCORES)], axis=0)
    return y[:, None, :].astype(np.float32)


# revision 12
# speedup vs baseline: 1.5865x; 1.1890x over previous
"""DiffFDN Trainium2 kernel, v4: all-SBUF fp16 history + gpsimd local_scatter.

Per core (4 items, lockstep): the 48000-step FDN scan runs as 94 blocks of
L=504 timesteps. History lives entirely in SBUF as a doubled ring of 16
slots ([128, 8064] fp16, rows 0-63 nxt series, 64-67 the y series; slot s
holds block s mod 8, duplicated at slot+8 so any <=5-slot window is a
contiguous column slice). Realignment of the 16 per-line delay taps uses
InstLocalScatter (per-partition int16 index tables, Pool engine): per block
pair {b, b+1} one BIG scatter assembles everything sourced from stages
<= b-2 into SB [128,1008] and one SMALL scatter adds stage b-1's
contribution for block b+1 into SS [128,504]. Both outputs are zero-filled
off their written lanes, so two accumulating matmuls (lhsT^T @ SB-half +
lhsT^T @ SS) reconstruct the exact tap matrix by linearity. No DMA on the
recurrence chain; y drains from ring rows 64-67 via periodic HWDGE DMAs.
"""

import numpy as np

SR = 48000
IR_LEN = 48000
DELAYS = [1009, 1123, 1231, 1321, 1433, 1543, 1657, 1777, 1879, 1987,
          2081, 2179, 2287, 2383, 2503, 2617]
N = 16
FEAT = 256
BATCH = 32
NCORES = 8
IPC = BATCH // NCORES          # items per core
L = 504
NBLK = 96                      # blocks 0,1 are identically zero (d_min=1009)
RING = 8                       # ring slots (doubled to 16 in storage)
RW = RING * L                  # 4032, primary ring width
NTAP = 3                       # longest delay lines fed by direct matmul taps
TAPS = list(range(N - NTAP, N))
DMAX = DELAYS[N - NTAP - 1]    # largest delay still handled by BIG (2287)
BIGLO = DMAX + 1               # 2288: BIG window [n0-BIGLO, n0-504), even
BIGW = BIGLO - L               # 1784
BIGOFF = 5 * L - BIGLO         # 232: window start within slot b-5
SMALLW = L                     # SMALL window [n0-504, n0) = stage b-1 exactly
M_OUT = IPC * N + IPC          # 68 psum rows (64 nxt + 4 y)

_BUILT = None


def _expm64(M):
    M = M.astype(np.float64)
    nrm = np.linalg.norm(M, ord=np.inf)
    k = max(0, int(np.ceil(np.log2(max(nrm, 1e-30)))) + 2)
    Ms = M / (2.0 ** k)
    E = np.eye(M.shape[0]) + Ms
    term = Ms.copy()
    for i in range(2, 18):
        term = term @ Ms / i
        E = E + term
    for _ in range(k):
        E = E @ E
    return E


def _prologue(x, WA, bA, WB, bB, WC, bC):
    x = np.asarray(x, np.float32)
    feat = x.mean(axis=1)
    A = np.tanh(feat @ np.asarray(WA).T + bA).reshape(-1, N, N)
    Bv = np.tanh(feat @ np.asarray(WB).T + bB)
    Cv = np.tanh(feat @ np.asarray(WC).T + bC)
    S = np.triu(A, 1)
    S = S - np.swapaxes(S, -1, -2)
    g = 10.0 ** (-3.0 / SR)
    G = g ** np.asarray(DELAYS, np.float64)
    A_g = np.stack([_expm64(S[b]) for b in range(S.shape[0])])
    A_g = (A_g * G[None, None, :]).astype(np.float32)
    return A_g, Bv.astype(np.float32), Cv.astype(np.float32)


def _core_inputs(A_g4, Bv4, Cv4):
    """lhsT [64, 68] fp16: contraction row r=4i+j (line i item j) ->
    psum rows m=4i'+j (nxt) and m=64+j (y)."""
    lhsT = np.zeros((IPC * N, M_OUT), np.float32)
    bv = np.zeros((IPC * N, 1), np.float32)
    for j in range(IPC):
        for i in range(N):
            r = 4 * i + j
            for ip in range(N):
                lhsT[r, 4 * ip + j] = A_g4[j, ip, i]
            lhsT[r, IPC * N + j] = Cv4[j, i]
            bv[r, 0] = Bv4[j, i]
    # per-tap weights: full 64-row operand with only that line's rows live
    # (matmul base partition must be 0/32/64, so taps contract all 64 rows)
    lhsTt = np.zeros((IPC * N, NTAP * M_OUT), np.float32)
    for t_i, i in enumerate(TAPS):
        rows = slice(4 * i, 4 * i + 4)
        lhsTt[rows, t_i * M_OUT:(t_i + 1) * M_OUT] = lhsT[rows, :]
    return lhsT.astype(np.float16), lhsTt.astype(np.float16), bv.astype(np.float16)


def _idx_tables():
    """Per-partition scatter tables (block-phase independent).

    BIG: data col k <-> time t = n0-BIGLO+k; row r (line i_r) valid when
    dst = k - BIGLO + d_r lands in [0, 1008) (covers block b cols 0-503 and
    block b+1 cols 504-1007, all sourced from stages <= b-2; the window's
    2B base must be 4B-aligned for the Q7 uint32-pair reads). Tap lines are
    excluded (handled by direct matmuls on ring slices).
    SMALL: data col k <-> t = n0-504+k (stage b-1); dst = k + d_r - 1008
    in [0, 504) covers block b+1's tail."""
    idxb = np.full((128, BIGW), -1, np.int16)
    idxs = np.full((128, SMALLW), -1, np.int16)
    for i in range(N):
        d = DELAYS[i]
        if i in TAPS:
            continue
        for j in range(IPC):
            r = 4 * i + j
            for k in range(BIGW):
                dst = k - BIGLO + d
                if 0 <= dst < 2 * L:
                    idxb[r, k] = dst
            for k in range(SMALLW):
                dst = k + d - 1008
                if 0 <= dst < L:
                    idxs[r, k] = dst
    return idxb, idxs


def _build():
    global _BUILT
    if _BUILT is not None:
        return _BUILT
    import concourse.bacc as bacc
    import concourse.mybir as mybir
    import concourse.tile as tile

    fp32 = mybir.dt.float32
    fp16 = mybir.dt.float16
    i16 = mybir.dt.int16
    nc = bacc.Bacc("TRN2", target_bir_lowering=False, debug=False)
    lhsT_d = nc.dram_tensor("lhsT", [IPC * N, M_OUT], fp16, kind="ExternalInput")
    lhsTt_d = nc.dram_tensor("lhsTt", [IPC * N, NTAP * M_OUT], fp16, kind="ExternalInput")
    bv_d = nc.dram_tensor("bv", [IPC * N, 1], fp16, kind="ExternalInput")
    idxb_d = nc.dram_tensor("idxb", [128, BIGW], i16, kind="ExternalInput")
    idxs_d = nc.dram_tensor("idxs", [128, SMALLW], i16, kind="ExternalInput")
    y_d = nc.dram_tensor("y", [IPC, NBLK * L], fp16, kind="ExternalOutput")

    with tile.TileContext(nc) as tc:
        with tc.tile_pool(name="const", bufs=1) as cpool, \
             tc.tile_pool(name="sb", bufs=4) as sbpool, \
             tc.tile_pool(name="ss", bufs=4) as sspool, \
             tc.tile_pool(name="ps", bufs=6, space="PSUM") as ppool:
            lhsT = cpool.tile([IPC * N, M_OUT], fp16)
            nc.sync.dma_start(lhsT[:, :], lhsT_d[:, :])
            lhsTt = cpool.tile([IPC * N, NTAP * M_OUT], fp16)
            nc.sync.dma_start(lhsTt[:, :], lhsTt_d[:, :])
            idxb = cpool.tile([128, BIGW], i16)
            nc.sync.dma_start(idxb[:, :], idxb_d[:, :])
            idxs = cpool.tile([128, SMALLW], i16)
            nc.sync.dma_start(idxs[:, :], idxs_d[:, :])

            ring = cpool.tile([128, 2 * RW], fp16)
            nc.vector.memset(ring[:, 0:RW], 0.0)
            nc.gpsimd.memset(ring[:, RW:2 * RW], 0.0)
            # impulse: nxt(0) = Bv at t=0 (slot 0 col 0, both images)
            nc.sync.dma_start(ring[0:IPC * N, 0:1], bv_d[:, :])
            nc.sync.dma_start(ring[0:IPC * N, RW:RW + 1], bv_d[:, :])
            # y for blocks 0,1 is identically zero
            nc.sync.dma_start(y_d[:, 0:2 * L], ring[IPC * N:M_OUT, 0:2 * L])

            for b in range(2, NBLK):
                n0 = b * L
                ps = ppool.tile([M_OUT, L], fp32)
                if b % 2 == 0:
                    # BIG scatter for pair {b, b+1}: window starts at
                    # slot (b-5) col BIGOFF  <->  t = n0 - BIGLO (4B-aligned)
                    base = ((b - 5) % RING) * L + BIGOFF
                    SB = sbpool.tile([128, 2 * L], fp16)
                    nc.gpsimd.local_scatter(
                        SB[:, :], ring[:, base:base + BIGW], idxb[:, :],
                        channels=128, num_elems=2 * L, num_idxs=BIGW)
                    nc.tensor.matmul(ps[:, :], lhsT[:, :], SB[0:IPC * N, 0:L],
                                     start=True, stop=False)
                else:
                    # SMALL scatter: data = ring slot (b-2) = stage b-1
                    # exactly (<-> t in [n0'-504, n0') for the pair's n0')
                    base = ((b - 2) % RING) * L
                    SS = sspool.tile([128, L], fp16)
                    nc.gpsimd.local_scatter(
                        SS[:, :], ring[:, base:base + SMALLW], idxs[:, :],
                        channels=128, num_elems=L, num_idxs=SMALLW)
                    nc.tensor.matmul(ps[:, :], lhsT[:, :], SB[0:IPC * N, L:2 * L],
                                     start=True, stop=False)
                    nc.tensor.matmul(ps[:, :], lhsT[:, :], SS[0:IPC * N, :],
                                     start=False, stop=False)
                # direct taps: longest lines read time-aligned ring slices
                for t_i, i in enumerate(TAPS):
                    d = DELAYS[i]
                    lo = n0 - d
                    sl = (lo // L) % RING
                    off = lo - (lo // L) * L
                    tb = sl * L + off
                    nc.tensor.matmul(
                        ps[:, :], lhsTt[:, t_i * M_OUT:(t_i + 1) * M_OUT],
                        ring[0:IPC * N, tb:tb + L],
                        start=False, stop=(t_i == NTAP - 1))
                # write block into both ring images (fp32 -> fp16 casts);
                # DVE (fast) writes the image the +2 consumer reads.
                slot = (b % RING) * L
                if b % 2 == 1 or b % RING >= 3:
                    fast, slow = slot, RW + slot
                else:
                    fast, slow = RW + slot, slot
                nc.vector.tensor_copy(ring[0:M_OUT, fast:fast + L], ps[:, :])
                nc.scalar.copy(ring[0:M_OUT, slow:slow + L], ps[:, :])
                # y drain: blocks [b-4, b-1] once their ring writes landed
                if b % 4 == 2 and b >= 6:
                    ybase = ((b - 4) % RING) * L
                    nc.sync.dma_start(
                        y_d[:, (b - 4) * L:b * L],
                        ring[IPC * N:M_OUT, ybase:ybase + 4 * L])
            # tail: blocks 94, 95 (ring slots 6, 7)
            nc.sync.dma_start(
                y_d[:, (NBLK - 2) * L:NBLK * L],
                ring[IPC * N:M_OUT, 6 * L:8 * L])
    nc.compile()
    _BUILT = nc
    return nc


def make_in_maps(inputs):
    A_g, Bv, Cv = _prologue(**inputs)
    idxb, idxs = _idx_tables()
    in_maps = []
    for k in range(NCORES):
        sl = slice(k * IPC, (k + 1) * IPC)
        lhsT, lhsTt, bv = _core_inputs(A_g[sl], Bv[sl], Cv[sl])
        in_maps.append({"lhsT": lhsT, "lhsTt": lhsTt, "bv": bv,
                        "idxb": idxb, "idxs": idxs})
    return in_maps


def kernel(x, WA, bA, WB, bB, WC, bC):
    from concourse import bass_utils

    in_maps = make_in_maps(
        {"x": x, "WA": WA, "bA": bA, "WB": WB, "bB": bB, "WC": WC, "bC": bC})
    nc = _build()
    res = bass_utils.run_bass_kernel_spmd(nc, in_maps, core_ids=list(range(NCORES)))
    y = np.concatenate(
        [res.results[k]["y"][:, :IR_LEN] for k in range(NCORES)], axis=0)
    return y[:, None, :].astype(np.float32)


# revision 13
# speedup vs baseline: 1.7471x; 1.1012x over previous
"""DiffFDN Trainium2 kernel, v4: all-SBUF fp16 history + gpsimd local_scatter.

Per core (4 items, lockstep): the 48000-step FDN scan runs as 94 blocks of
L=504 timesteps. History lives entirely in SBUF as a doubled ring of 16
slots ([128, 8064] fp16, rows 0-63 nxt series, 64-67 the y series; slot s
holds block s mod 8, duplicated at slot+8 so any <=5-slot window is a
contiguous column slice). Realignment of the 16 per-line delay taps uses
InstLocalScatter (per-partition int16 index tables, Pool engine): per block
pair {b, b+1} one BIG scatter assembles everything sourced from stages
<= b-2 into SB [128,1008] and one SMALL scatter adds stage b-1's
contribution for block b+1 into SS [128,504]. Both outputs are zero-filled
off their written lanes, so two accumulating matmuls (lhsT^T @ SB-half +
lhsT^T @ SS) reconstruct the exact tap matrix by linearity. No DMA on the
recurrence chain; y drains from ring rows 64-67 via periodic HWDGE DMAs.
"""

import numpy as np

SR = 48000
IR_LEN = 48000
DELAYS = [1009, 1123, 1231, 1321, 1433, 1543, 1657, 1777, 1879, 1987,
          2081, 2179, 2287, 2383, 2503, 2617]
N = 16
FEAT = 256
BATCH = 32
NCORES = 8
IPC = BATCH // NCORES          # items per core
L = 504
NBLK = 96                      # blocks 0,1 are identically zero (d_min=1009)
RING = 8                       # ring slots (doubled to 16 in storage)
RW = RING * L                  # 4032, primary ring width
NTAP = 5                       # longest delay lines fed by direct matmul taps
TAPS = list(range(N - NTAP, N))
DMAX = DELAYS[N - NTAP - 1]    # largest delay still handled by BIG (2287)
BIGLO = DMAX + 1               # 2288: BIG window [n0-BIGLO, n0-504), even
BIGW = BIGLO - L               # 1784
BIGOFF = 5 * L - BIGLO         # 232: window start within slot b-5
SMALLW = L                     # SMALL window [n0-504, n0) = stage b-1 exactly
M_OUT = IPC * N + IPC          # 68 psum rows (64 nxt + 4 y)

_BUILT = None


def _expm64(M):
    M = M.astype(np.float64)
    nrm = np.linalg.norm(M, ord=np.inf)
    k = max(0, int(np.ceil(np.log2(max(nrm, 1e-30)))) + 2)
    Ms = M / (2.0 ** k)
    E = np.eye(M.shape[0]) + Ms
    term = Ms.copy()
    for i in range(2, 18):
        term = term @ Ms / i
        E = E + term
    for _ in range(k):
        E = E @ E
    return E


def _prologue(x, WA, bA, WB, bB, WC, bC):
    x = np.asarray(x, np.float32)
    feat = x.mean(axis=1)
    A = np.tanh(feat @ np.asarray(WA).T + bA).reshape(-1, N, N)
    Bv = np.tanh(feat @ np.asarray(WB).T + bB)
    Cv = np.tanh(feat @ np.asarray(WC).T + bC)
    S = np.triu(A, 1)
    S = S - np.swapaxes(S, -1, -2)
    g = 10.0 ** (-3.0 / SR)
    G = g ** np.asarray(DELAYS, np.float64)
    A_g = np.stack([_expm64(S[b]) for b in range(S.shape[0])])
    A_g = (A_g * G[None, None, :]).astype(np.float32)
    return A_g, Bv.astype(np.float32), Cv.astype(np.float32)


def _core_inputs(A_g4, Bv4, Cv4):
    """lhsT [64, 68] fp16: contraction row r=4i+j (line i item j) ->
    psum rows m=4i'+j (nxt) and m=64+j (y)."""
    lhsT = np.zeros((IPC * N, M_OUT), np.float32)
    bv = np.zeros((IPC * N, 1), np.float32)
    for j in range(IPC):
        for i in range(N):
            r = 4 * i + j
            for ip in range(N):
                lhsT[r, 4 * ip + j] = A_g4[j, ip, i]
            lhsT[r, IPC * N + j] = Cv4[j, i]
            bv[r, 0] = Bv4[j, i]
    # per-tap weights: full 64-row operand with only that line's rows live
    # (matmul base partition must be 0/32/64, so taps contract all 64 rows)
    lhsTt = np.zeros((IPC * N, NTAP * M_OUT), np.float32)
    for t_i, i in enumerate(TAPS):
        rows = slice(4 * i, 4 * i + 4)
        lhsTt[rows, t_i * M_OUT:(t_i + 1) * M_OUT] = lhsT[rows, :]
    return lhsT.astype(np.float16), lhsTt.astype(np.float16), bv.astype(np.float16)


def _idx_tables():
    """Per-partition scatter tables (block-phase independent).

    BIG: data col k <-> time t = n0-BIGLO+k; row r (line i_r) valid when
    dst = k - BIGLO + d_r lands in [0, 1008) (covers block b cols 0-503 and
    block b+1 cols 504-1007, all sourced from stages <= b-2; the window's
    2B base must be 4B-aligned for the Q7 uint32-pair reads). Tap lines are
    excluded (handled by direct matmuls on ring slices).
    SMALL: data col k <-> t = n0-504+k (stage b-1); dst = k + d_r - 1008
    in [0, 504) covers block b+1's tail."""
    idxb = np.full((128, BIGW), -1, np.int16)
    idxs = np.full((128, SMALLW), -1, np.int16)
    for i in range(N):
        d = DELAYS[i]
        if i in TAPS:
            continue
        for j in range(IPC):
            r = 4 * i + j
            for k in range(BIGW):
                dst = k - BIGLO + d
                if 0 <= dst < 2 * L:
                    idxb[r, k] = dst
            for k in range(SMALLW):
                dst = k + d - 1008
                if 0 <= dst < L:
                    idxs[r, k] = dst
    return idxb, idxs


def _build():
    global _BUILT
    if _BUILT is not None:
        return _BUILT
    import concourse.bacc as bacc
    import concourse.mybir as mybir
    import concourse.tile as tile

    fp32 = mybir.dt.float32
    fp16 = mybir.dt.float16
    i16 = mybir.dt.int16
    nc = bacc.Bacc("TRN2", target_bir_lowering=False, debug=False)
    lhsT_d = nc.dram_tensor("lhsT", [IPC * N, M_OUT], fp16, kind="ExternalInput")
    lhsTt_d = nc.dram_tensor("lhsTt", [IPC * N, NTAP * M_OUT], fp16, kind="ExternalInput")
    bv_d = nc.dram_tensor("bv", [IPC * N, 1], fp16, kind="ExternalInput")
    idxb_d = nc.dram_tensor("idxb", [128, BIGW], i16, kind="ExternalInput")
    idxs_d = nc.dram_tensor("idxs", [128, SMALLW], i16, kind="ExternalInput")
    y_d = nc.dram_tensor("y", [IPC, NBLK * L], fp16, kind="ExternalOutput")

    with tile.TileContext(nc) as tc:
        with tc.tile_pool(name="const", bufs=1) as cpool, \
             tc.tile_pool(name="sb", bufs=4) as sbpool, \
             tc.tile_pool(name="ss", bufs=4) as sspool, \
             tc.tile_pool(name="ps", bufs=6, space="PSUM") as ppool:
            lhsT = cpool.tile([IPC * N, M_OUT], fp16)
            nc.sync.dma_start(lhsT[:, :], lhsT_d[:, :])
            lhsTt = cpool.tile([IPC * N, NTAP * M_OUT], fp16)
            nc.sync.dma_start(lhsTt[:, :], lhsTt_d[:, :])
            idxb = cpool.tile([128, BIGW], i16)
            nc.sync.dma_start(idxb[:, :], idxb_d[:, :])
            idxs = cpool.tile([128, SMALLW], i16)
            nc.sync.dma_start(idxs[:, :], idxs_d[:, :])

            ring = cpool.tile([128, 2 * RW], fp16)
            nc.vector.memset(ring[:, 0:RW], 0.0)
            nc.gpsimd.memset(ring[:, RW:2 * RW], 0.0)
            # impulse: nxt(0) = Bv at t=0 (slot 0 col 0, both images)
            nc.sync.dma_start(ring[0:IPC * N, 0:1], bv_d[:, :])
            nc.sync.dma_start(ring[0:IPC * N, RW:RW + 1], bv_d[:, :])
            # y for blocks 0,1 is identically zero
            nc.sync.dma_start(y_d[:, 0:2 * L], ring[IPC * N:M_OUT, 0:2 * L])

            for b in range(2, NBLK):
                n0 = b * L
                ps = ppool.tile([M_OUT, L], fp32)
                # direct taps first: they read old ring slices, so they
                # run under the scatter and stay off the critical chain
                for t_i, i in enumerate(TAPS):
                    d = DELAYS[i]
                    lo = n0 - d
                    sl = (lo // L) % RING
                    off = lo - (lo // L) * L
                    tb = sl * L + off
                    nc.tensor.matmul(
                        ps[:, :], lhsTt[:, t_i * M_OUT:(t_i + 1) * M_OUT],
                        ring[0:IPC * N, tb:tb + L],
                        start=(t_i == 0), stop=False)
                if b % 2 == 0:
                    # BIG scatter for pair {b, b+1}: window starts at
                    # slot (b-5) col BIGOFF  <->  t = n0 - BIGLO (4B-aligned)
                    base = ((b - 5) % RING) * L + BIGOFF
                    SB = sbpool.tile([128, 2 * L], fp16)
                    nc.gpsimd.local_scatter(
                        SB[:, :], ring[:, base:base + BIGW], idxb[:, :],
                        channels=128, num_elems=2 * L, num_idxs=BIGW)
                    nc.tensor.matmul(ps[:, :], lhsT[:, :], SB[0:IPC * N, 0:L],
                                     start=False, stop=True)
                else:
                    # SMALL scatter: data = ring slot (b-2) = stage b-1
                    # exactly (<-> t in [n0'-504, n0') for the pair's n0')
                    base = ((b - 2) % RING) * L
                    SS = sspool.tile([128, L], fp16)
                    nc.gpsimd.local_scatter(
                        SS[:, :], ring[:, base:base + SMALLW], idxs[:, :],
                        channels=128, num_elems=L, num_idxs=SMALLW)
                    nc.tensor.matmul(ps[:, :], lhsT[:, :], SS[0:IPC * N, :],
                                     start=False, stop=False)
                    nc.tensor.matmul(ps[:, :], lhsT[:, :], SB[0:IPC * N, L:2 * L],
                                     start=False, stop=True)
                # write block into both ring images (fp32 -> fp16 casts);
                # the image the +2 consumer reads is written split across
                # DVE and ACT in parallel to shorten the chain.
                slot = (b % RING) * L
                if b % 2 == 1 or b % RING >= 3:
                    fast, slow = slot, RW + slot
                else:
                    fast, slow = RW + slot, slot
                SPL = 364
                nc.vector.tensor_copy(ring[0:M_OUT, fast:fast + SPL], ps[:, 0:SPL])
                nc.scalar.copy(ring[0:M_OUT, fast + SPL:fast + L], ps[:, SPL:L])
                nc.scalar.copy(ring[0:M_OUT, slow:slow + L], ps[:, :])
                # y drain: blocks [b-4, b-1] once their ring writes landed
                if b % 4 == 2 and b >= 6:
                    ybase = ((b - 4) % RING) * L
                    nc.sync.dma_start(
                        y_d[:, (b - 4) * L:b * L],
                        ring[IPC * N:M_OUT, ybase:ybase + 4 * L])
            # tail: blocks 94, 95 (ring slots 6, 7)
            nc.sync.dma_start(
                y_d[:, (NBLK - 2) * L:NBLK * L],
                ring[IPC * N:M_OUT, 6 * L:8 * L])
    nc.compile()
    _BUILT = nc
    return nc


def make_in_maps(inputs):
    A_g, Bv, Cv = _prologue(**inputs)
    idxb, idxs = _idx_tables()
    in_maps = []
    for k in range(NCORES):
        sl = slice(k * IPC, (k + 1) * IPC)
        lhsT, lhsTt, bv = _core_inputs(A_g[sl], Bv[sl], Cv[sl])
        in_maps.append({"lhsT": lhsT, "lhsTt": lhsTt, "bv": bv,
                        "idxb": idxb, "idxs": idxs})
    return in_maps


def kernel(x, WA, bA, WB, bB, WC, bC):
    from concourse import bass_utils

    in_maps = make_in_maps(
        {"x": x, "WA": WA, "bA": bA, "WB": WB, "bB": bB, "WC": WC, "bC": bC})
    nc = _build()
    res = bass_utils.run_bass_kernel_spmd(nc, in_maps, core_ids=list(range(NCORES)))
    y = np.concatenate(
        [res.results[k]["y"][:, :IR_LEN] for k in range(NCORES)], axis=0)
    return y[:, None, :].astype(np.float32)


# revision 14
# speedup vs baseline: 2.0321x; 1.1631x over previous
"""DiffFDN Trainium2 kernel, v4: all-SBUF fp16 history + gpsimd local_scatter.

Per core (4 items, lockstep): the 48000-step FDN scan runs as 94 blocks of
L=504 timesteps. History lives entirely in SBUF as a doubled ring of 16
slots ([128, 8064] fp16, rows 0-63 nxt series, 64-67 the y series; slot s
holds block s mod 8, duplicated at slot+8 so any <=5-slot window is a
contiguous column slice). Realignment of the 16 per-line delay taps uses
InstLocalScatter (per-partition int16 index tables, Pool engine): per block
pair {b, b+1} one BIG scatter assembles everything sourced from stages
<= b-2 into SB [128,1008] and one SMALL scatter adds stage b-1's
contribution for block b+1 into SS [128,504]. Both outputs are zero-filled
off their written lanes, so two accumulating matmuls (lhsT^T @ SB-half +
lhsT^T @ SS) reconstruct the exact tap matrix by linearity. No DMA on the
recurrence chain; y drains from ring rows 64-67 via periodic HWDGE DMAs.
"""

import numpy as np

SR = 48000
IR_LEN = 48000
DELAYS = [1009, 1123, 1231, 1321, 1433, 1543, 1657, 1777, 1879, 1987,
          2081, 2179, 2287, 2383, 2503, 2617]
N = 16
FEAT = 256
BATCH = 32
NCORES = 8
IPC = BATCH // NCORES          # items per core
L = 504
NBLK = 96                      # blocks 0,1 are identically zero (d_min=1009)
RING = 8                       # ring slots (doubled to 16 in storage)
RW = RING * L                  # 4032, primary ring width
NTAP = 6                       # longest delay lines fed by direct matmul taps
TAPS = list(range(N - NTAP, N))
DMAX = DELAYS[N - NTAP - 1]    # largest delay still handled by BIG (2287)
BIGLO = DMAX + 1               # 2288: BIG window [n0-BIGLO, n0-504), even
BIGW = BIGLO - L               # 1784
BIGOFF = 5 * L - BIGLO         # 232: window start within slot b-5
SMALLW = L                     # SMALL window [n0-504, n0) = stage b-1 exactly
M_OUT = IPC * N + IPC          # 68 psum rows (64 nxt + 4 y)

_BUILT = None


def _expm64(M):
    M = M.astype(np.float64)
    nrm = np.linalg.norm(M, ord=np.inf)
    k = max(0, int(np.ceil(np.log2(max(nrm, 1e-30)))) + 2)
    Ms = M / (2.0 ** k)
    E = np.eye(M.shape[0]) + Ms
    term = Ms.copy()
    for i in range(2, 18):
        term = term @ Ms / i
        E = E + term
    for _ in range(k):
        E = E @ E
    return E


def _prologue(x, WA, bA, WB, bB, WC, bC):
    x = np.asarray(x, np.float32)
    feat = x.mean(axis=1)
    A = np.tanh(feat @ np.asarray(WA).T + bA).reshape(-1, N, N)
    Bv = np.tanh(feat @ np.asarray(WB).T + bB)
    Cv = np.tanh(feat @ np.asarray(WC).T + bC)
    S = np.triu(A, 1)
    S = S - np.swapaxes(S, -1, -2)
    g = 10.0 ** (-3.0 / SR)
    G = g ** np.asarray(DELAYS, np.float64)
    A_g = np.stack([_expm64(S[b]) for b in range(S.shape[0])])
    A_g = (A_g * G[None, None, :]).astype(np.float32)
    return A_g, Bv.astype(np.float32), Cv.astype(np.float32)


def _core_inputs(A_g4, Bv4, Cv4):
    """lhsT [64, 68] fp16: contraction row r=4i+j (line i item j) ->
    psum rows m=4i'+j (nxt) and m=64+j (y)."""
    lhsT = np.zeros((IPC * N, M_OUT), np.float32)
    bv = np.zeros((IPC * N, 1), np.float32)
    for j in range(IPC):
        for i in range(N):
            r = 4 * i + j
            for ip in range(N):
                lhsT[r, 4 * ip + j] = A_g4[j, ip, i]
            lhsT[r, IPC * N + j] = Cv4[j, i]
            bv[r, 0] = Bv4[j, i]
    # per-tap weights: full 64-row operand with only that line's rows live
    # (matmul base partition must be 0/32/64, so taps contract all 64 rows)
    lhsTt = np.zeros((IPC * N, NTAP * M_OUT), np.float32)
    for t_i, i in enumerate(TAPS):
        rows = slice(4 * i, 4 * i + 4)
        lhsTt[rows, t_i * M_OUT:(t_i + 1) * M_OUT] = lhsT[rows, :]
    return lhsT.astype(np.float16), lhsTt.astype(np.float16), bv.astype(np.float16)


def _idx_tables():
    """Per-partition scatter tables (block-phase independent).

    BIG: data col k <-> time t = n0-BIGLO+k; row r (line i_r) valid when
    dst = k - BIGLO + d_r lands in [0, 1008) (covers block b cols 0-503 and
    block b+1 cols 504-1007, all sourced from stages <= b-2; the window's
    2B base must be 4B-aligned for the Q7 uint32-pair reads). Tap lines are
    excluded (handled by direct matmuls on ring slices).
    SMALL: data col k <-> t = n0-504+k (stage b-1); dst = k + d_r - 1008
    in [0, 504) covers block b+1's tail."""
    idxb = np.full((128, BIGW), -1, np.int16)
    idxs = np.full((128, SMALLW), -1, np.int16)
    for i in range(N):
        d = DELAYS[i]
        if i in TAPS:
            continue
        for j in range(IPC):
            r = 4 * i + j
            for k in range(BIGW):
                dst = k - BIGLO + d
                if 0 <= dst < 2 * L:
                    idxb[r, k] = dst
            for k in range(SMALLW):
                dst = k + d - 1008
                if 0 <= dst < L:
                    idxs[r, k] = dst
    return idxb, idxs


def _build():
    global _BUILT
    if _BUILT is not None:
        return _BUILT
    import concourse.bacc as bacc
    import concourse.mybir as mybir
    import concourse.tile as tile

    fp32 = mybir.dt.float32
    fp16 = mybir.dt.float16
    i16 = mybir.dt.int16
    nc = bacc.Bacc("TRN2", target_bir_lowering=False, debug=False)
    lhsT_d = nc.dram_tensor("lhsT", [IPC * N, M_OUT], fp16, kind="ExternalInput")
    lhsTt_d = nc.dram_tensor("lhsTt", [IPC * N, NTAP * M_OUT], fp16, kind="ExternalInput")
    bv_d = nc.dram_tensor("bv", [IPC * N, 1], fp16, kind="ExternalInput")
    idxb_d = nc.dram_tensor("idxb", [128, BIGW], i16, kind="ExternalInput")
    idxs_d = nc.dram_tensor("idxs", [128, SMALLW], i16, kind="ExternalInput")
    y_d = nc.dram_tensor("y", [IPC, NBLK * L], fp16, kind="ExternalOutput")

    with tile.TileContext(nc) as tc:
        with tc.tile_pool(name="const", bufs=1) as cpool, \
             tc.tile_pool(name="sb", bufs=4) as sbpool, \
             tc.tile_pool(name="ss", bufs=4) as sspool, \
             tc.tile_pool(name="ps", bufs=6, space="PSUM") as ppool:
            lhsT = cpool.tile([IPC * N, M_OUT], fp16)
            nc.sync.dma_start(lhsT[:, :], lhsT_d[:, :])
            lhsTt = cpool.tile([IPC * N, NTAP * M_OUT], fp16)
            nc.sync.dma_start(lhsTt[:, :], lhsTt_d[:, :])
            idxb = cpool.tile([128, BIGW], i16)
            nc.sync.dma_start(idxb[:, :], idxb_d[:, :])
            idxs = cpool.tile([128, SMALLW], i16)
            nc.sync.dma_start(idxs[:, :], idxs_d[:, :])

            ring = cpool.tile([128, 2 * RW], fp16)
            nc.vector.memset(ring[:, 0:RW], 0.0)
            nc.gpsimd.memset(ring[:, RW:2 * RW], 0.0)
            # impulse: nxt(0) = Bv at t=0 (slot 0 col 0, both images)
            nc.sync.dma_start(ring[0:IPC * N, 0:1], bv_d[:, :])
            nc.sync.dma_start(ring[0:IPC * N, RW:RW + 1], bv_d[:, :])
            # y for blocks 0,1 is identically zero
            nc.sync.dma_start(y_d[:, 0:2 * L], ring[IPC * N:M_OUT, 0:2 * L])

            for b in range(2, NBLK):
                n0 = b * L
                ps = ppool.tile([M_OUT, L], fp32)
                # direct taps first: they read old ring slices, so they
                # run under the scatter and stay off the critical chain
                for t_i, i in enumerate(TAPS):
                    d = DELAYS[i]
                    lo = n0 - d
                    sl = (lo // L) % RING
                    off = lo - (lo // L) * L
                    tb = sl * L + off
                    nc.tensor.matmul(
                        ps[:, :], lhsTt[:, t_i * M_OUT:(t_i + 1) * M_OUT],
                        ring[0:IPC * N, tb:tb + L],
                        start=(t_i == 0), stop=False)
                if b % 2 == 0:
                    # BIG scatter for pair {b, b+1}: window starts at
                    # slot (b-5) col BIGOFF  <->  t = n0 - BIGLO (4B-aligned)
                    base = ((b - 5) % RING) * L + BIGOFF
                    SB = sbpool.tile([128, 2 * L], fp16)
                    nc.gpsimd.local_scatter(
                        SB[:, :], ring[:, base:base + BIGW], idxb[:, :],
                        channels=128, num_elems=2 * L, num_idxs=BIGW)
                    nc.tensor.matmul(ps[:, :], lhsT[:, :], SB[0:IPC * N, 0:L],
                                     start=False, stop=True)
                else:
                    # SMALL scatter: data = ring slot (b-2) = stage b-1
                    # exactly (<-> t in [n0'-504, n0') for the pair's n0')
                    base = ((b - 2) % RING) * L
                    SS = sspool.tile([128, L], fp16)
                    nc.gpsimd.local_scatter(
                        SS[:, :], ring[:, base:base + SMALLW], idxs[:, :],
                        channels=128, num_elems=L, num_idxs=SMALLW)
                    nc.tensor.matmul(ps[:, :], lhsT[:, :], SS[0:IPC * N, :],
                                     start=False, stop=False)
                    nc.tensor.matmul(ps[:, :], lhsT[:, :], SB[0:IPC * N, L:2 * L],
                                     start=False, stop=True)
                # write block into both ring images (fp32 -> fp16 casts);
                # the image the +2 consumer reads is written split across
                # DVE and ACT in parallel to shorten the chain.
                slot = (b % RING) * L
                if b % 2 == 1 or b % RING >= 3:
                    fast, slow = slot, RW + slot
                else:
                    fast, slow = RW + slot, slot
                nc.vector.tensor_copy(ring[0:M_OUT, fast:fast + L], ps[:, :])
                nc.scalar.copy(ring[0:M_OUT, slow:slow + L], ps[:, :])
                # y drain: blocks [b-4, b-1] once their ring writes landed
                if b % 4 == 2 and b >= 6:
                    ybase = ((b - 4) % RING) * L
                    nc.sync.dma_start(
                        y_d[:, (b - 4) * L:b * L],
                        ring[IPC * N:M_OUT, ybase:ybase + 4 * L])
            # tail: blocks 94, 95 (ring slots 6, 7)
            nc.sync.dma_start(
                y_d[:, (NBLK - 2) * L:NBLK * L],
                ring[IPC * N:M_OUT, 6 * L:8 * L])
    nc.compile()
    _BUILT = nc
    return nc


def make_in_maps(inputs):
    A_g, Bv, Cv = _prologue(**inputs)
    idxb, idxs = _idx_tables()
    in_maps = []
    for k in range(NCORES):
        sl = slice(k * IPC, (k + 1) * IPC)
        lhsT, lhsTt, bv = _core_inputs(A_g[sl], Bv[sl], Cv[sl])
        in_maps.append({"lhsT": lhsT, "lhsTt": lhsTt, "bv": bv,
                        "idxb": idxb, "idxs": idxs})
    return in_maps


def kernel(x, WA, bA, WB, bB, WC, bC):
    from concourse import bass_utils

    in_maps = make_in_maps(
        {"x": x, "WA": WA, "bA": bA, "WB": WB, "bB": bB, "WC": WC, "bC": bC})
    nc = _build()
    res = bass_utils.run_bass_kernel_spmd(nc, in_maps, core_ids=list(range(NCORES)))
    y = np.concatenate(
        [res.results[k]["y"][:, :IR_LEN] for k in range(NCORES)], axis=0)
    return y[:, None, :].astype(np.float32)


# revision 18
# speedup vs baseline: 2.0332x; 1.0006x over previous
"""DiffFDN Trainium2 kernel, v4: all-SBUF fp16 history + gpsimd local_scatter.

Per core (4 items, lockstep): the 48000-step FDN scan runs as 94 blocks of
L=504 timesteps. History lives entirely in SBUF as a doubled ring of 16
slots ([128, 8064] fp16, rows 0-63 nxt series, 64-67 the y series; slot s
holds block s mod 8, duplicated at slot+8 so any <=5-slot window is a
contiguous column slice). Realignment of the 16 per-line delay taps uses
InstLocalScatter (per-partition int16 index tables, Pool engine): per block
pair {b, b+1} one BIG scatter assembles everything sourced from stages
<= b-2 into SB [128,1008] and one SMALL scatter adds stage b-1's
contribution for block b+1 into SS [128,504]. Both outputs are zero-filled
off their written lanes, so two accumulating matmuls (lhsT^T @ SB-half +
lhsT^T @ SS) reconstruct the exact tap matrix by linearity. No DMA on the
recurrence chain; y drains from ring rows 64-67 via periodic HWDGE DMAs.
"""

import numpy as np

SR = 48000
IR_LEN = 48000
DELAYS = [1009, 1123, 1231, 1321, 1433, 1543, 1657, 1777, 1879, 1987,
          2081, 2179, 2287, 2383, 2503, 2617]
N = 16
FEAT = 256
BATCH = 32
NCORES = 8
IPC = BATCH // NCORES          # items per core
L = 504
NBLK = 96                      # blocks 0,1 are identically zero (d_min=1009)
RING = 8                       # ring slots (doubled to 16 in storage)
RW = RING * L                  # 4032, primary ring width
NTAP = 6                       # longest delay lines fed by direct matmul taps
TAPS = list(range(N - NTAP, N))
DMAX = DELAYS[N - NTAP - 1]    # largest delay still handled by BIG (2287)
BIGLO = DMAX + 1               # 2288: BIG window [n0-BIGLO, n0-504), even
BIGW = BIGLO - L               # 1784
BIGOFF = 5 * L - BIGLO         # 232: window start within slot b-5
SMALLW = L                     # SMALL window [n0-504, n0) = stage b-1 exactly
M_OUT = IPC * N + IPC          # 68 psum rows (64 nxt + 4 y)

_BUILT = None


def _expm64(M):
    M = M.astype(np.float64)
    nrm = np.linalg.norm(M, ord=np.inf)
    k = max(0, int(np.ceil(np.log2(max(nrm, 1e-30)))) + 2)
    Ms = M / (2.0 ** k)
    E = np.eye(M.shape[0]) + Ms
    term = Ms.copy()
    for i in range(2, 18):
        term = term @ Ms / i
        E = E + term
    for _ in range(k):
        E = E @ E
    return E


def _prologue(x, WA, bA, WB, bB, WC, bC):
    x = np.asarray(x, np.float32)
    feat = x.mean(axis=1)
    A = np.tanh(feat @ np.asarray(WA).T + bA).reshape(-1, N, N)
    Bv = np.tanh(feat @ np.asarray(WB).T + bB)
    Cv = np.tanh(feat @ np.asarray(WC).T + bC)
    S = np.triu(A, 1)
    S = S - np.swapaxes(S, -1, -2)
    g = 10.0 ** (-3.0 / SR)
    G = g ** np.asarray(DELAYS, np.float64)
    A_g = np.stack([_expm64(S[b]) for b in range(S.shape[0])])
    A_g = (A_g * G[None, None, :]).astype(np.float32)
    return A_g, Bv.astype(np.float32), Cv.astype(np.float32)


def _core_inputs(A_g4, Bv4, Cv4):
    """lhsT [64, 68] fp16: contraction row r=4i+j (line i item j) ->
    psum rows m=4i'+j (nxt) and m=64+j (y)."""
    lhsT = np.zeros((IPC * N, M_OUT), np.float32)
    bv = np.zeros((IPC * N, 1), np.float32)
    for j in range(IPC):
        for i in range(N):
            r = 4 * i + j
            for ip in range(N):
                lhsT[r, 4 * ip + j] = A_g4[j, ip, i]
            lhsT[r, IPC * N + j] = Cv4[j, i]
            bv[r, 0] = Bv4[j, i]
    # per-tap weights: full 64-row operand with only that line's rows live
    # (matmul base partition must be 0/32/64, so taps contract all 64 rows)
    lhsTt = np.zeros((IPC * N, NTAP * M_OUT), np.float32)
    for t_i, i in enumerate(TAPS):
        rows = slice(4 * i, 4 * i + 4)
        lhsTt[rows, t_i * M_OUT:(t_i + 1) * M_OUT] = lhsT[rows, :]
    return lhsT.astype(np.float16), lhsTt.astype(np.float16), bv.astype(np.float16)


def _idx_tables():
    """Per-partition scatter tables (block-phase independent).

    BIG: data col k <-> time t = n0-BIGLO+k; row r (line i_r) valid when
    dst = k - BIGLO + d_r lands in [0, 1008) (covers block b cols 0-503 and
    block b+1 cols 504-1007, all sourced from stages <= b-2; the window's
    2B base must be 4B-aligned for the Q7 uint32-pair reads). Tap lines are
    excluded (handled by direct matmuls on ring slices).
    SMALL: data col k <-> t = n0-504+k (stage b-1); dst = k + d_r - 1008
    in [0, 504) covers block b+1's tail."""
    idxb = np.full((128, BIGW), -1, np.int16)
    idxs = np.full((128, SMALLW), -1, np.int16)
    for i in range(N):
        d = DELAYS[i]
        if i in TAPS:
            continue
        for j in range(IPC):
            r = 4 * i + j
            for k in range(BIGW):
                dst = k - BIGLO + d
                if 0 <= dst < 2 * L:
                    idxb[r, k] = dst
            for k in range(SMALLW):
                dst = k + d - 1008
                if 0 <= dst < L:
                    idxs[r, k] = dst
    return idxb, idxs


def _build():
    global _BUILT
    if _BUILT is not None:
        return _BUILT
    import concourse.bacc as bacc
    import concourse.mybir as mybir
    import concourse.tile as tile

    fp32 = mybir.dt.float32
    fp16 = mybir.dt.float16
    i16 = mybir.dt.int16
    nc = bacc.Bacc("TRN2", target_bir_lowering=False, debug=False)
    lhsT_d = nc.dram_tensor("lhsT", [IPC * N, M_OUT], fp16, kind="ExternalInput")
    lhsTt_d = nc.dram_tensor("lhsTt", [IPC * N, NTAP * M_OUT], fp16, kind="ExternalInput")
    bv_d = nc.dram_tensor("bv", [IPC * N, 1], fp16, kind="ExternalInput")
    idxb_d = nc.dram_tensor("idxb", [128, BIGW], i16, kind="ExternalInput")
    idxs_d = nc.dram_tensor("idxs", [128, SMALLW], i16, kind="ExternalInput")
    y_d = nc.dram_tensor("y", [IPC, NBLK * L], fp16, kind="ExternalOutput")

    with tile.TileContext(nc) as tc:
        with tc.tile_pool(name="const", bufs=1) as cpool, \
             tc.tile_pool(name="sb", bufs=4) as sbpool, \
             tc.tile_pool(name="ss", bufs=4) as sspool, \
             tc.tile_pool(name="ps", bufs=6, space="PSUM") as ppool:
            lhsT = cpool.tile([IPC * N, M_OUT], fp16)
            nc.sync.dma_start(lhsT[:, :], lhsT_d[:, :])
            lhsTt = cpool.tile([IPC * N, NTAP * M_OUT], fp16)
            nc.sync.dma_start(lhsTt[:, :], lhsTt_d[:, :])
            idxb = cpool.tile([128, BIGW], i16)
            nc.scalar.dma_start(idxb[:, :], idxb_d[:, :])
            idxs = cpool.tile([128, SMALLW], i16)
            nc.sync.dma_start(idxs[:, :], idxs_d[:, :])

            ring = cpool.tile([128, 2 * RW], fp16)
            nc.vector.memset(ring[:, 0:RW], 0.0)
            nc.gpsimd.memset(ring[:, RW:2 * RW], 0.0)
            # impulse: nxt(0) = Bv at t=0 (slot 0 col 0, both images)
            nc.sync.dma_start(ring[0:IPC * N, 0:1], bv_d[:, :])
            nc.sync.dma_start(ring[0:IPC * N, RW:RW + 1], bv_d[:, :])
            # y for blocks 0,1 is identically zero
            nc.sync.dma_start(y_d[:, 0:2 * L], ring[IPC * N:M_OUT, 0:2 * L])

            for b in range(2, NBLK):
                n0 = b * L
                ps = ppool.tile([M_OUT, L], fp32)
                # direct taps first: they read old ring slices, so they
                # run under the scatter and stay off the critical chain
                for t_i, i in enumerate(TAPS):
                    d = DELAYS[i]
                    lo = n0 - d
                    sl = (lo // L) % RING
                    off = lo - (lo // L) * L
                    tb = sl * L + off
                    nc.tensor.matmul(
                        ps[:, :], lhsTt[:, t_i * M_OUT:(t_i + 1) * M_OUT],
                        ring[0:IPC * N, tb:tb + L],
                        start=(t_i == 0), stop=False)
                if b % 2 == 0:
                    # BIG scatter for pair {b, b+1}: window starts at
                    # slot (b-5) col BIGOFF  <->  t = n0 - BIGLO (4B-aligned)
                    base = ((b - 5) % RING) * L + BIGOFF
                    SB = sbpool.tile([128, 2 * L], fp16)
                    nc.gpsimd.local_scatter(
                        SB[:, :], ring[:, base:base + BIGW], idxb[:, :],
                        channels=128, num_elems=2 * L, num_idxs=BIGW)
                    nc.tensor.matmul(ps[:, :], lhsT[:, :], SB[0:IPC * N, 0:L],
                                     start=False, stop=True)
                else:
                    # SMALL scatter: data = ring slot (b-2) = stage b-1
                    # exactly (<-> t in [n0'-504, n0') for the pair's n0')
                    base = ((b - 2) % RING) * L
                    SS = sspool.tile([128, L], fp16)
                    nc.gpsimd.local_scatter(
                        SS[:, :], ring[:, base:base + SMALLW], idxs[:, :],
                        channels=128, num_elems=L, num_idxs=SMALLW)
                    nc.tensor.matmul(ps[:, :], lhsT[:, :], SS[0:IPC * N, :],
                                     start=False, stop=False)
                    nc.tensor.matmul(ps[:, :], lhsT[:, :], SB[0:IPC * N, L:2 * L],
                                     start=False, stop=True)
                # write block into both ring images (fp32 -> fp16 casts);
                # the image the +2 consumer reads is written split across
                # DVE and ACT in parallel to shorten the chain.
                slot = (b % RING) * L
                if b % 2 == 1 or b % RING >= 3:
                    fast, slow = slot, RW + slot
                else:
                    fast, slow = RW + slot, slot
                nc.vector.tensor_copy(ring[0:M_OUT, fast:fast + L], ps[:, :])
                nc.scalar.copy(ring[0:M_OUT, slow:slow + L], ps[:, :])
                # y drain: blocks [b-4, b-1] once their ring writes landed
                if b % 4 == 2 and b >= 6:
                    ybase = ((b - 4) % RING) * L
                    nc.sync.dma_start(
                        y_d[:, (b - 4) * L:b * L],
                        ring[IPC * N:M_OUT, ybase:ybase + 4 * L])
            # tail: blocks 94, 95 (ring slots 6, 7)
            nc.sync.dma_start(
                y_d[:, (NBLK - 2) * L:NBLK * L],
                ring[IPC * N:M_OUT, 6 * L:8 * L])
    nc.compile()
    _BUILT = nc
    return nc


def make_in_maps(inputs):
    A_g, Bv, Cv = _prologue(**inputs)
    idxb, idxs = _idx_tables()
    in_maps = []
    for k in range(NCORES):
        sl = slice(k * IPC, (k + 1) * IPC)
        lhsT, lhsTt, bv = _core_inputs(A_g[sl], Bv[sl], Cv[sl])
        in_maps.append({"lhsT": lhsT, "lhsTt": lhsTt, "bv": bv,
                        "idxb": idxb, "idxs": idxs})
    return in_maps


def kernel(x, WA, bA, WB, bB, WC, bC):
    from concourse import bass_utils

    in_maps = make_in_maps(
        {"x": x, "WA": WA, "bA": bA, "WB": WB, "bB": bB, "WC": WC, "bC": bC})
    nc = _build()
    res = bass_utils.run_bass_kernel_spmd(nc, in_maps, core_ids=list(range(NCORES)))
    y = np.concatenate(
        [res.results[k]["y"][:, :IR_LEN] for k in range(NCORES)], axis=0)
    return y[:, None, :].astype(np.float32)
